# revision 1
# baseline (speedup 1.0000x reference)
"""Trainium2 Bass kernel for nn_Pooling_block (B=128, N=785, C=384, pp=2).

Pure data-parallel over batch: 16 batches per core x 8 NeuronCores.

v3 design (memory-regime; DMA floor ~135us/core):
  - All DRAM inputs declared float32r (same bits as f32) so every load runs
    on HWDGE with no cast DMAs and feeds PE matmuls at full f32r rate.
  - x host-pre-gathered to patch-major [B, 196, 4C]; per batch two loads:
    patches 0:128 -> [128, 4C] (one 6 KB descriptor per partition, engages
    all 16 SDMA engines evenly) and patches 128:196 -> [68, 4C].
  - edge folded [128, 4C] (rows 0:512) + [64, 4C] at base partition 64
    (rows 512:768) + [17, C]: shifts edge bytes onto partitions 64-127 to
    balance per-partition DMA load against G (which is heavier on 0-67).
  - per-batch sums -> sigmoid -> s_row; s columns collected per GROUP of 4
    into one [128, 3, 4] PSUM tile; ci = s @ W_lin.T per batch (3 matmuls),
    broadcast to 128 partitions via gpsimd partition_broadcast.
  - scores: fused DVE scalar_tensor_tensor in f32 (no cast passes).
  - pooled emitted in bf16 -> bf16 PE transposes -> bf16 final matmuls
    against W_out_cls.T (K=384 fp32 accumulation; ~2e-3 rel err, gate 2e-2).
  - loads issue from SP only (its stream has no compute to block); stores
    from ACT right after its own PSUM->SBUF copies.
  - PSUM: 8 banks (es, ns, scolT, cip, tp2 x2, fo x2).
"""
import os
import sys

sys.path.insert(0, "/opt/trn_rl_repo")

import numpy as np

import concourse.bass as bass
import concourse.tile as tile
from concourse import bacc, mybir
from concourse.bass_utils import run_bass_kernel_spmd

B, N, C = 128, 785, 384
HW = N - 1          # 784
H = 28              # grid side
HP = 14             # pooled grid side
NPATCH = HP * HP    # 196
NB = 16             # batches per core
NCORES = 8
NOUT = 1 + NPATCH   # 197
CO = 2 * C          # 768
GRP = 4             # batches per chain group
NGRP = NB // GRP
R1, R2 = 128, 68    # patch split

F32 = mybir.dt.float32
F32R = mybir.dt.float32r
BF16 = mybir.dt.bfloat16
ADD = mybir.AluOpType.add
MUL = mybir.AluOpType.mult
SIGMOID = mybir.ActivationFunctionType.Sigmoid


def build_program(w_scalars):
    """Build the per-core SPMD program. w_scalars = (w00, w01, w10, w11) when
    the per-patch weights are channel-uniform, else None (general path)."""
    nc = bacc.Bacc(None, target_bir_lowering=False, debug=False)

    # x is host-pre-gathered patch-major and zero-padded to 256 patches so
    # BOTH per-batch loads span 128 partitions (sub-128-partition HWDGE
    # transfers concentrate on 4 of the 16 SDMA engines).
    x_d = nc.declare_dram_parameter("x", [NB, 256, 4 * C], F32R, isOutput=False)
    e_d = nc.declare_dram_parameter("edge", [NB, N, C], F32R, isOutput=False)
    wlt_d = nc.declare_dram_parameter("wlt", [C, C], F32R, isOutput=False)
    wct_d = nc.declare_dram_parameter("wct", [C, CO], F32, isOutput=False)
    id_d = nc.declare_dram_parameter("ident", [128, 128], F32R, isOutput=False)
    clsc_d = nc.declare_dram_parameter("cls_cm", [128, 3, NB], F32, isOutput=False)
    if w_scalars is None:
        wqr_d = nc.declare_dram_parameter("wqr", [4, 128, C], F32, isOutput=False)
    out_d = nc.declare_dram_parameter("out", [NB, NOUT, CO], F32, isOutput=True)

    uniform_w = w_scalars is not None and len(set(w_scalars)) == 1
    # (tile index, rows, acm column offset)
    SPLITS = ((0, R1, 1), (1, R2, 1 + R1))

    with tile.TileContext(nc) as tc:
        with (
            tc.tile_pool(name="const", bufs=1) as cpool,
            tc.tile_pool(name="gx", bufs=6) as gxp,
            tc.tile_pool(name="ed", bufs=2) as edp,
            tc.tile_pool(name="apool", bufs=5) as ap,
            tc.tile_pool(name="work", bufs=2) as wk,
            tc.tile_pool(name="small", bufs=2) as sm,
            tc.tile_pool(name="cibp", bufs=1) as cibp,
            tc.tile_pool(name="acm", bufs=2) as acmp,
            tc.tile_pool(name="ost", bufs=2) as ostp,
            tc.tile_pool(name="psE", bufs=2, space="PSUM") as psE,
            tc.tile_pool(name="psC", bufs=1, space="PSUM") as psC,
            tc.tile_pool(name="psT", bufs=1, space="PSUM") as psT,
            tc.tile_pool(name="psF", bufs=2, space="PSUM") as psF,
        ):
            # ---- constants ----
            ones_f = cpool.tile([128, 1], F32)
            nc.vector.memset(ones_f[:], 1.0)
            ones_r = cpool.tile([128, 1], F32R)
            nc.vector.tensor_copy(ones_r[:], ones_f[:])

            ident_r = cpool.tile([128, 128], F32R)
            nc.sync.dma_start(ident_r[:], id_d[:])
            ident_bf = cpool.tile([128, 128], BF16)
            nc.vector.tensor_copy(ident_bf[:], ident_r[:])

            wlt_r = []
            for cch in range(3):
                t = cpool.tile([128, C], F32R, tag=f"wlt{cch}")
                nc.sync.dma_start(t[:], wlt_d[128 * cch : 128 * (cch + 1), :])
                wlt_r.append(t)

            wct_bf = []
            for cch in range(3):
                stg = ostp.tile([128, CO], F32, tag="ost0")
                nc.sync.dma_start(stg[:], wct_d[128 * cch : 128 * (cch + 1), :])
                t = cpool.tile([128, CO], BF16, tag=f"wct{cch}")
                nc.vector.tensor_copy(t[:], stg[:])
                wct_bf.append(t)

            if w_scalars is None:
                wqr_t = []
                for k in range(4):
                    t = cpool.tile([128, C], F32, tag=f"wqr{k}")
                    nc.sync.dma_start(t[:], wqr_d[k])
                    wqr_t.append(t)

            cls_cm = cpool.tile([128, 3, NB], F32)
            nc.sync.dma_start(cls_cm[:], clsc_d[:])
            cls_bf = cpool.tile([128, 3, NB], BF16)
            nc.vector.tensor_copy(cls_bf[:], cls_cm[:])

            # small first group fills the pipeline sooner (scores gate on the
            # group chain); small last group drains sooner
            group_list = [range(0, 2), range(2, 6), range(6, 10),
                          range(10, 14), range(14, 16)]
            for bs in group_list:
                glen = len(bs)
                g_t, a_t = {}, {}

                # -- sub-loop 1: loads + per-batch token sums --
                # one PSUM bank holds both the s-column collection (cols
                # 0:3*GRP) and the per-batch ci row (cols 3*GRP:3*GRP+C)
                chain_ps = psC.tile([128, 3 * GRP + C], F32, tag="chain")
                scolT = chain_ps[:, 0 : 3 * GRP].rearrange("p (c g) -> p c g", g=GRP)
                for b in bs:
                    gb = b - bs[0]
                    # all loads issue from SP (sync): its stream has no
                    # compute, so slot waits never block compute instructions.
                    g1 = gxp.tile([R1, 4, C], F32R, tag="g1")
                    nc.sync.dma_start(
                        g1[:], x_d[b, 0:128, :].rearrange("p (k c) -> p k c", k=4)
                    )
                    # g2 loads 128 padded patches; compute uses rows 0:68 only
                    g2f = gxp.tile([128, 4, C], F32R, tag="g2")
                    nc.sync.dma_start(
                        g2f[:], x_d[b, 128:256, :].rearrange("p (k c) -> p k c", k=4)
                    )
                    g_t[(b, 0)], g_t[(b, 1)] = g1, g2f

                    efold = edp.tile([128, 6 * C], F32R, tag="efold")
                    nc.sync.dma_start(
                        efold[:],
                        e_d[b, 0:768, :].rearrange("(p k) c -> p (k c)", p=128),
                    )
                    etl = edp.tile([17, C], F32R, tag="etl")
                    nc.gpsimd.dma_start(etl[:], e_d[b, 768:785, :])

                    # edge sums
                    es = psE.tile([1, C], F32, tag="es")
                    for k in range(6):
                        nc.tensor.matmul(
                            es[:], ones_r[:], efold[:, C * k : C * (k + 1)],
                            start=(k == 0), stop=False,
                        )
                    nc.tensor.matmul(
                        es[:], ones_r[0:17, :], etl[:], start=False, stop=True
                    )

                    # vertical pair sums A_q (f32r, one fused DVE op per tile)
                    ns = psE.tile([1, C], F32, tag="ns")
                    first = True
                    for t_i, rn, _ in SPLITS:
                        at = ap.tile([rn, 2, C], F32R, tag=f"a{t_i}")
                        nc.vector.tensor_add(
                            at[:],
                            g_t[(b, t_i)][0:rn, 0:2, :],
                            g_t[(b, t_i)][0:rn, 2:4, :],
                        )
                        a_t[(b, t_i)] = at
                    for q in range(2):
                        for t_i, rn, _ in SPLITS:
                            nc.tensor.matmul(
                                ns[:], ones_r[0:rn, :], a_t[(b, t_i)][:, q, :],
                                start=first, stop=(q == 1 and t_i == 1),
                            )
                            first = False

                    # sigmoid means -> s_row; transpose into column gb of scolT
                    se = sm.tile([1, C], F32, tag="se")
                    nc.scalar.activation(se[:], es[:], SIGMOID, scale=1.0 / N)
                    sn = sm.tile([1, C], F32, tag="sn")
                    nc.scalar.activation(sn[:], ns[:], SIGMOID, scale=1.0 / HW)
                    s_row = sm.tile([1, C], F32, tag="srow")
                    nc.vector.tensor_add(s_row[:], se[:], sn[:])
                    for cch in range(3):
                        nc.tensor.matmul(
                            scolT[:, cch, gb : gb + 1],
                            s_row[:, 128 * cch : 128 * (cch + 1)],
                            ones_f[0:1, :],
                            start=True, stop=True,
                        )

                # -- group chain: s columns -> ci rows -> broadcasts up front --
                scolT_sb = sm.tile([128, 3, GRP], F32R, tag="scolsb")
                nc.scalar.copy(scolT_sb[:], scolT[:])

                cibs = []
                for gb in range(glen):
                    cip = chain_ps[0:1, 3 * GRP : 3 * GRP + C]
                    for cch in range(3):
                        nc.tensor.matmul(
                            cip[:], scolT_sb[:, cch, gb : gb + 1], wlt_r[cch][:],
                            start=(cch == 0), stop=(cch == 2),
                        )
                    ci_b = sm.tile([1, C], F32R, tag=f"cirow{gb}")
                    nc.scalar.copy(ci_b[:], cip[:])
                    cib = cibp.tile([128, C], F32R, tag=f"cib{gb}")
                    nc.gpsimd.partition_broadcast(cib[:], ci_b[:])
                    cibs.append(cib)

                # -- sub-loop 2: scores / pooled / transpose / final / store --
                for b in bs:
                    gb = b - bs[0]
                    cib = cibs[gb]

                    sig = {}
                    for t_i, rn, _ in SPLITS:
                        sa = sm.tile([rn, 4], F32, tag=f"sacc{t_i}")
                        for k in range(4):
                            scr = wk.tile([rn, C], BF16, tag=f"scr{t_i}")
                            nc.vector.scalar_tensor_tensor(
                                scr[:], g_t[(b, t_i)][0:rn, k, :], 1.0, cib[0:rn, :],
                                MUL, MUL, accum_out=sa[:, k : k + 1],
                            )
                        sg = sm.tile([rn, 4], F32, tag=f"sig{t_i}")
                        nc.scalar.activation(sg[:], sa[:], SIGMOID)
                        sig[t_i] = sg

                    pooled = {}
                    for t_i, rn, _ in SPLITS:
                        at = a_t[(b, t_i)]
                        if uniform_w:
                            # t01[:, q] = (sig[2q]+1) + (sig[2q+1]+1), w folded
                            w00 = w_scalars[0]
                            t01 = sm.tile([rn, 2], F32, tag=f"t01_{t_i}")
                            nc.vector.tensor_add(
                                t01[:], sig[t_i][:, 0:4:2], sig[t_i][:, 1:4:2]
                            )
                            if w00 != 1.0:
                                # (t01 + 2) * w00 == t01*w00 + 2*w00
                                nc.vector.tensor_scalar(
                                    t01[:], t01[:], w00, 2.0 * w00, MUL, ADD
                                )
                            else:
                                nc.vector.tensor_scalar_add(t01[:], t01[:], 2.0)
                            p0 = wk.tile([rn, C], F32, tag=f"p0_{t_i}")
                            nc.vector.tensor_scalar_mul(p0[:], at[:, 0, :], t01[:, 0:1])
                            pl = wk.tile([rn, C], BF16, tag=f"pl{t_i}")
                            nc.vector.scalar_tensor_tensor(
                                pl[:], at[:, 1, :], t01[:, 1:2], p0[:], MUL, ADD
                            )
                        elif w_scalars is not None:
                            sp = sm.tile([rn, 4], F32, tag=f"sp{t_i}")
                            nc.vector.tensor_scalar_add(sp[:], sig[t_i][:], 1.0)
                            wrow = sm.tile([rn, 4], F32, tag=f"wrow{t_i}")
                            for k in range(4):
                                nc.vector.memset(
                                    wrow[:, k : k + 1], float(w_scalars[k])
                                )
                            nc.vector.tensor_mul(sp[:], sp[:], wrow[:])
                            t01 = sm.tile([rn, 2], F32, tag=f"t01_{t_i}")
                            nc.vector.tensor_add(t01[:], sp[:, 0:4:2], sp[:, 1:4:2])
                            p0 = wk.tile([rn, C], F32, tag=f"p0_{t_i}")
                            nc.vector.tensor_scalar_mul(p0[:], at[:, 0, :], t01[:, 0:1])
                            pl = wk.tile([rn, C], BF16, tag=f"pl{t_i}")
                            nc.vector.scalar_tensor_tensor(
                                pl[:], at[:, 1, :], t01[:, 1:2], p0[:], MUL, ADD
                            )
                        else:
                            sp = sm.tile([rn, 4], F32, tag=f"sp{t_i}")
                            nc.vector.tensor_scalar_add(sp[:], sig[t_i][:], 1.0)
                            # general per-channel weights: m_q[rn, C], combine
                            mqs = []
                            for q in range(2):
                                m0 = wk.tile([rn, C], F32, tag=f"mq{q}{t_i}a")
                                nc.vector.tensor_scalar_mul(
                                    m0[:], wqr_t[2 * q][0:rn, :],
                                    sp[:, 2 * q : 2 * q + 1],
                                )
                                mq = wk.tile([rn, C], F32, tag=f"mq{q}{t_i}b")
                                nc.vector.scalar_tensor_tensor(
                                    mq[:], wqr_t[2 * q + 1][0:rn, :],
                                    sp[:, 2 * q + 1 : 2 * q + 2], m0[:], MUL, ADD,
                                )
                                mqs.append(mq)
                            p0 = wk.tile([rn, C], F32, tag=f"p0_{t_i}")
                            nc.vector.tensor_mul(p0[:], at[:, 0, :], mqs[0][:])
                            p1 = wk.tile([rn, C], F32, tag=f"p1_{t_i}")
                            nc.vector.tensor_mul(p1[:], at[:, 1, :], mqs[1][:])
                            pl = wk.tile([rn, C], BF16, tag=f"pl{t_i}")
                            nc.vector.tensor_add(pl[:], p0[:], p1[:])
                        pooled[t_i] = pl

                    # c-major A via bf16 PE transposes + cls column
                    a_cm = []
                    for cch in range(3):
                        tp2 = psT.tile([128, NPATCH], BF16, tag="tp2")
                        for t_i, rn, col in SPLITS:
                            nc.tensor.transpose(
                                tp2[:, col - 1 : col - 1 + rn],
                                pooled[t_i][:, 128 * cch : 128 * (cch + 1)],
                                ident_bf[0:rn, 0:rn],
                            )
                        acm = acmp.tile([128, NOUT], BF16, tag=f"acm{cch}")
                        nc.scalar.copy(acm[:, 0:1], cls_bf[:, cch, b : b + 1])
                        nc.scalar.copy(acm[:, 1:NOUT], tp2[:])
                        a_cm.append(acm)

                    # final matmul: out[row, co] = A_cm.T @ W_out_cls.T (bf16)
                    for rch, (r0, rn) in enumerate(((0, 128), (128, 69))):
                        stile = ostp.tile([128, CO], F32, tag=f"ost{rch}")
                        for nh in range(2):
                            fo = psF.tile([128, C], F32, tag="fo")
                            for cch in range(3):
                                nc.tensor.matmul(
                                    fo[0:rn, :],
                                    a_cm[cch][:, r0 : r0 + rn],
                                    wct_bf[cch][:, C * nh : C * (nh + 1)],
                                    start=(cch == 0), stop=(cch == 2),
                                )
                            # both halves on ACT so the store that follows in
                            # ACT's stream never waits on another engine
                            nc.scalar.copy(
                                stile[0:rn, C * nh : C * (nh + 1)], fo[0:rn, :]
                            )
                        # stores via SWDGE: its descriptor swizzle spreads
                        # partitions across all 16 SDMA engines
                        nc.gpsimd.dma_start(out_d[b, r0 : r0 + rn, :], stile[0:rn, :])

    nc.compile()
    return nc


def prepare(x, edge, W_lin, W_out_cls, weights):
    """Host-side prep shared by kernel() and the timing harness: returns
    (w_scalars, in_maps)."""
    x = np.ascontiguousarray(x, dtype=np.float32)
    edge = np.ascontiguousarray(edge, dtype=np.float32)
    # pre-gather nodes to patch-major [B, 196, 4C] (slot = 2p+q), zero-padded
    # to 256 patches so both per-batch loads span 128 partitions
    xg = np.zeros((B, 256, 4 * C), dtype=np.float32)
    xg[:, :NPATCH] = (
        x[:, 1:, :]
        .reshape(B, HP, 2, HP, 2, C)
        .transpose(0, 1, 3, 2, 4, 5)
        .reshape(B, NPATCH, 4 * C)
    )
    wlt = np.ascontiguousarray(np.asarray(W_lin).T, dtype=np.float32)
    wct = np.ascontiguousarray(np.asarray(W_out_cls).T, dtype=np.float32)
    w = np.asarray(weights, dtype=np.float32)

    c_uniform = bool(np.all(w == w[0:1]))
    w_scalars = tuple(float(v) for v in w[0].reshape(4)) if c_uniform else None

    ident = np.eye(128, dtype=np.float32)
    in_maps = []
    for core in range(NCORES):
        sl = slice(core * NB, (core + 1) * NB)
        cls_cm = np.ascontiguousarray(
            x[sl, 0, :].T.reshape(3, 128, NB).transpose(1, 0, 2), dtype=np.float32
        )
        m = {
            "x": xg[sl], "edge": edge[sl], "wlt": wlt, "wct": wct, "ident": ident,
            "cls_cm": cls_cm,
        }
        if w_scalars is None:
            wqr = np.empty((4, 128, C), dtype=np.float32)
            for q in range(2):
                for r in range(2):
                    wqr[2 * q + r] = np.broadcast_to(w[:, q, r], (128, C))
            m["wqr"] = wqr
        in_maps.append(m)
    return w_scalars, in_maps


def kernel(x, edge, W_lin, W_out_cls, weights):
    w_scalars, in_maps = prepare(x, edge, W_lin, W_out_cls, weights)
    nc = build_program(w_scalars)
    res = run_bass_kernel_spmd(nc, in_maps, list(range(NCORES)))
    out = np.concatenate([r["out"] for r in res.results], axis=0)
    return out



# revision 4
# speedup vs baseline: 1.4256x; 1.4256x over previous
"""Trainium2 Bass kernel for nn_Pooling_block (B=128, N=785, C=384, pp=2).

Pure data-parallel over batch: 16 batches per core x 8 NeuronCores.

v4 design ("c-major", fp16 inputs):
  - All large inputs sent to the device in float16 (x quantization error
    ~2^-11 -> end-to-end rel err ~1e-3, gate 2e-2) halving HBM read traffic
    vs f32: per-core reads ~20 MB, writes 9.7 MB, floor ~83us @ 358 GB/s.
  - x host-pre-gathered CHANNEL-major: [B, 128, 3, 784]; partition = channel
    mod 128, free = (chunk, patch, quad).  edge likewise [B, 128, 3, 785].
    No padding, one 588 KB HWDGE load each per batch.
  - c-major eliminates all per-batch PE transposes and big scalar copies:
      * edge/node sums: DVE tensor_reduce along free axis -> [128, 3] cols
      * s = sig(es)+sig(ns) lands directly as c-major columns of sG
      * group ci: 9 matmuls (wlt blocks stationary) -> ciG [128, 3, G]
      * scores: 6 small matmuls (ci column stationary, nodes moving)
      * t01 broadcast to 128 partitions via a 1-row ones matmul into PSUM
      * pooled: 2 DVE ops/chunk writing fp16 straight into acm (the final
        matmul's stationary operand)
  - finals: 12 bf16-rate fp16 matmuls vs W_out_cls.T chunks (K=384 fp32
    accumulation), PSUM->SBUF copies on ACT, stores on ACT's own HWDGE
    ring (qActDynamicHW) so loads (sync ring) and stores don't serialize.
"""
import os
import sys

sys.path.insert(0, "/opt/trn_rl_repo")

import numpy as np

import concourse.bass as bass
import concourse.tile as tile
from concourse import bacc, mybir
from concourse.bass_utils import run_bass_kernel_spmd

B, N, C = 128, 785, 384
HW = N - 1          # 784
H = 28              # grid side
HP = 14             # pooled grid side
NPATCH = HP * HP    # 196
NB = 16             # batches per core
NCORES = 8
NOUT = 1 + NPATCH   # 197
CO = 2 * C          # 768
NCH = 3             # channel chunks of 128
GRP = 4             # max batches per group

F32 = mybir.dt.float32
F16 = mybir.dt.float16
ADD = mybir.AluOpType.add
MUL = mybir.AluOpType.mult
SIGMOID = mybir.ActivationFunctionType.Sigmoid
AXC = mybir.AxisListType.X


def build_program(w_scalars):
    """Build the per-core SPMD program. w_scalars = (w00, w01, w10, w11) when
    the per-patch weights are channel-uniform, else None (general path)."""
    nc = bacc.Bacc(None, target_bir_lowering=False, debug=False)

    x_d = nc.declare_dram_parameter("x", [NB, 128, NCH, HW], F16, isOutput=False)
    e_d = nc.declare_dram_parameter("edge", [NB, 128, NCH, N], F16, isOutput=False)
    wlt_d = nc.declare_dram_parameter("wlt", [128, NCH, C], F16, isOutput=False)
    wct_d = nc.declare_dram_parameter("wct", [128, NCH, CO], F16, isOutput=False)
    clsc_d = nc.declare_dram_parameter("cls_cm", [128, NCH, NB], F16, isOutput=False)
    general_w = w_scalars is None
    if general_w:
        wq_d = nc.declare_dram_parameter("wq_cm", [128, NCH, 4], F32, isOutput=False)
    out_d = nc.declare_dram_parameter("out", [NB, NOUT, CO], F32, isOutput=True)

    uniform_w = (not general_w) and len(set(w_scalars)) == 1

    with tile.TileContext(nc) as tc:
        with (
            tc.tile_pool(name="const", bufs=1) as cpool,
            tc.tile_pool(name="gx", bufs=6) as gxp,
            tc.tile_pool(name="ed", bufs=3) as edp,
            tc.tile_pool(name="apool", bufs=6) as ap,
            tc.tile_pool(name="work", bufs=2) as wk,
            tc.tile_pool(name="small", bufs=2) as sm,
            tc.tile_pool(name="sgp", bufs=2) as sgp,
            tc.tile_pool(name="acm", bufs=6) as acmp,
            tc.tile_pool(name="ost", bufs=4) as ostp,
            tc.tile_pool(name="psS", bufs=2, space="PSUM") as psS,
            tc.tile_pool(name="psB", bufs=2, space="PSUM") as psB,
            tc.tile_pool(name="psC", bufs=1, space="PSUM") as psC,
            tc.tile_pool(name="psF", bufs=2, space="PSUM") as psF,
        ):
            # ---- constants ----
            ones_f = cpool.tile([1, 128], F32)
            nc.vector.memset(ones_f[:], 1.0)
            ones_h = cpool.tile([1, 128], F16)
            nc.vector.tensor_copy(ones_h[:], ones_f[:])

            wlt_h = cpool.tile([128, NCH, C], F16)
            nc.sync.dma_start(wlt_h[:], wlt_d[:])
            wct_h = cpool.tile([128, NCH, CO], F16)
            nc.sync.dma_start(wct_h[:], wct_d[:])
            cls_h = cpool.tile([128, NCH, NB], F16)
            nc.sync.dma_start(cls_h[:], clsc_d[:])
            if general_w:
                wq_h = cpool.tile([128, NCH, 4], F32)
                nc.sync.dma_start(wq_h[:], wq_d[:])

            group_list = [range(0, 2), range(2, 6), range(6, 10),
                          range(10, 14), range(14, 16)]
            for bs in group_list:
                glen = len(bs)
                x_t, a_t = {}, {}

                # s columns for the group, c-major: [128, chunk, batch]
                sG = sgp.tile([128, NCH, GRP], F16, tag="sG")

                # -- sub-loop 1: loads + per-batch token sums -> s columns --
                for b in bs:
                    gb = b - bs[0]
                    xb = gxp.tile([128, NCH, NPATCH, 4], F16, tag="xb")
                    nc.sync.dma_start(
                        xb[:],
                        x_d[b].rearrange("p k (n q) -> p k n q", q=4),
                    )
                    x_t[b] = xb
                    eb = edp.tile([128, NCH, N], F16, tag="eb")
                    nc.sync.dma_start(eb[:], e_d[b])

                    # vertical pair sums a[c, chunk, patch, j] (fp16)
                    at = ap.tile([128, NCH, NPATCH, 2], F16, tag="at")
                    nc.vector.tensor_add(
                        at[:], xb[:, :, :, 0:2], xb[:, :, :, 2:4]
                    )
                    a_t[b] = at

                    # free-axis reductions -> c-major sum columns [128, 3]
                    ns = sm.tile([128, NCH, 1], F32, tag="ns")
                    nc.vector.tensor_reduce(
                        ns[:, :, 0],
                        at[:].rearrange("p k n j -> p k (n j)"),
                        AXC, ADD,
                    )
                    es = sm.tile([128, NCH, 1], F32, tag="es")
                    nc.vector.tensor_reduce(es[:, :, 0], eb[:], AXC, ADD)

                    se = sm.tile([128, NCH, 1], F32, tag="se")
                    nc.scalar.activation(se[:], es[:], SIGMOID, scale=1.0 / N)
                    sn = sm.tile([128, NCH, 1], F32, tag="sn")
                    nc.scalar.activation(sn[:], ns[:], SIGMOID, scale=1.0 / HW)
                    nc.vector.tensor_add(sG[:, :, gb : gb + 1], se[:], sn[:])

                # -- group ci: 9 matmuls, all c-major, no transposes --
                ciG_ps = psC.tile([128, NCH, GRP], F32, tag="ciG")
                for m in range(NCH):
                    for k in range(NCH):
                        nc.tensor.matmul(
                            ciG_ps[:, m, 0:glen],
                            wlt_h[:, k, 128 * m : 128 * (m + 1)],
                            sG[:, k, 0:glen],
                            start=(k == 0), stop=(k == 2),
                        )
                ciG = sm.tile([128, NCH, GRP], F16, tag="ciG_sb")
                nc.scalar.copy(ciG[:], ciG_ps[:])

                # -- pass A: scores -> t01 -> pooled -> acm per batch --
                acms = {}
                for b in bs:
                    gb = b - bs[0]
                    xb, at = x_t[b], a_t[b]

                    sig = sm.tile([1, NPATCH, 2, 2], F32, tag="sig")
                    sigf = sig[:].rearrange("o n j r -> o (n j r)")
                    for h in range(2):
                        sc_ps = psS.tile([1, 392], F32, tag="sc")
                        for k in range(NCH):
                            nc.tensor.matmul(
                                sc_ps[:],
                                ciG[:, k, gb : gb + 1],
                                xb[:, k, 98 * h : 98 * (h + 1), :].rearrange(
                                    "p n q -> p (n q)"
                                ),
                                start=(k == 0), stop=(k == 2),
                            )
                        nc.scalar.activation(
                            sigf[:, 392 * h : 392 * (h + 1)], sc_ps[:], SIGMOID
                        )

                    acm = [
                        acmp.tile([128, NOUT], F16, tag=f"acm{cch}",
                                  name=f"acm{cch}")
                        for cch in range(NCH)
                    ]
                    for cch in range(NCH):
                        nc.scalar.copy(
                            acm[cch][:, 0:1], cls_h[:, cch, b : b + 1]
                        )
                    acms[b] = acm

                    if not general_w:
                        # t01[p, j] = sig(4p+2j) + sig(4p+2j+1); w2 folds the
                        # uniform/channel-uniform weights: (t01 + 2) * w
                        t01 = sm.tile([1, NPATCH, 2], F32, tag="t01")
                        nc.vector.tensor_reduce(
                            t01[:, :, :], sig[:], AXC, ADD
                        )
                        t01h = sm.tile([1, NPATCH, 2], F16, tag="t01h")
                        if uniform_w:
                            w00 = w_scalars[0]
                            nc.vector.tensor_scalar(
                                t01h[:], t01[:], w00, 2.0 * w00, MUL, ADD
                            )
                        else:
                            # channel-uniform but quad-varying weights:
                            # m[j] = w[2j]*(1+sig(2j-slot)) + w[2j+1]*(1+...)
                            # fold via two tensor_scalar ops on the halves
                            wv = w_scalars
                            tq = sm.tile([1, NPATCH, 2, 2], F32, tag="tq")
                            nc.vector.tensor_scalar_add(tq[:], sig[:], 1.0)
                            for j in range(2):
                                nc.vector.tensor_scalar(
                                    tq[:, :, j, 0:1], tq[:, :, j, 0:1],
                                    float(wv[2 * j]), None, MUL,
                                )
                                nc.vector.tensor_scalar(
                                    tq[:, :, j, 1:2], tq[:, :, j, 1:2],
                                    float(wv[2 * j + 1]), None, MUL,
                                )
                            nc.vector.tensor_reduce(
                                t01h[:, :, :], tq[:], AXC, ADD
                            )
                        # broadcast t01 to all 128 partitions via PE
                        t01b = psB.tile([128, NPATCH, 2], F32, tag="t01b")
                        nc.tensor.matmul(
                            t01b[:].rearrange("p n j -> p (n j)"),
                            ones_h[:],
                            t01h[:].rearrange("o n j -> o (n j)"),
                            start=True, stop=True,
                        )
                        # pooled[c, p] = a0*t01b0 + a1*t01b1, fp16 into acm
                        for cch in range(NCH):
                            prod = wk.tile([128, NPATCH, 2], F32, tag="prod")
                            nc.vector.tensor_mul(
                                prod[:], at[:, cch], t01b[:]
                            )
                            nc.vector.tensor_add(
                                acm[cch][:, 1:NOUT].rearrange("p (n u) -> p n u", u=1),
                                prod[:, :, 0:1], prod[:, :, 1:2],
                            )
                    else:
                        # fully general per-channel weights
                        sp1 = sm.tile([1, NPATCH, 2, 2], F16, tag="sp1")
                        nc.vector.tensor_scalar_add(sp1[:], sig[:], 1.0)
                        sp1f = sp1[:].rearrange("o n j r -> o (n j r)")
                        sp1s = wk.tile([128, NPATCH, 2, 2], F32, tag="sp1s")
                        sp1sf = sp1s[:].rearrange("p n j r -> p (n j r)")
                        for h in range(2):
                            spb = psB.tile([128, 392], F32, tag="t01b")
                            nc.tensor.matmul(
                                spb[:], ones_h[:],
                                sp1f[:, 392 * h : 392 * (h + 1)],
                                start=True, stop=True,
                            )
                            nc.scalar.copy(
                                sp1sf[:, 392 * h : 392 * (h + 1)], spb[:]
                            )
                        for cch in range(NCH):
                            mj = []
                            for j in range(2):
                                u = wk.tile([128, NPATCH], F32, tag=f"u{j}")
                                nc.vector.tensor_scalar_mul(
                                    u[:], sp1s[:, :, j, 0],
                                    wq_h[:, cch, 2 * j : 2 * j + 1],
                                )
                                m = wk.tile([128, NPATCH], F32, tag=f"m{j}")
                                nc.vector.scalar_tensor_tensor(
                                    m[:], sp1s[:, :, j, 1],
                                    wq_h[:, cch, 2 * j + 1 : 2 * j + 2],
                                    u[:], MUL, ADD,
                                )
                                mj.append(m)
                            p0 = wk.tile([128, NPATCH], F32, tag="p0")
                            nc.vector.tensor_mul(
                                p0[:], at[:, cch, :, 0], mj[0][:]
                            )
                            p1 = wk.tile([128, NPATCH], F32, tag="p1")
                            nc.vector.tensor_mul(
                                p1[:], at[:, cch, :, 1], mj[1][:]
                            )
                            nc.vector.tensor_add(
                                acm[cch][:, 1:NOUT], p0[:], p1[:]
                            )

                # -- pass B: finals + stores per batch --
                for b in bs:
                    acm = acms[b]
                    for r0, rn in ((0, 128), (128, 69)):
                        stile = ostp.tile([128, CO], F32, tag="ost")
                        for nh in range(2):
                            fo = psF.tile([128, C], F32, tag="fo")
                            for cch in range(NCH):
                                nc.tensor.matmul(
                                    fo[0:rn, :],
                                    acm[cch][:, r0 : r0 + rn],
                                    wct_h[:, cch, C * nh : C * (nh + 1)],
                                    start=(cch == 0), stop=(cch == 2),
                                )
                            nc.scalar.copy(
                                stile[0:rn, C * nh : C * (nh + 1)], fo[0:rn, :]
                            )
                        nc.scalar.dma_start(
                            out_d[b, r0 : r0 + rn, :], stile[0:rn, :]
                        )

    nc.compile()
    return nc


def prepare(x, edge, W_lin, W_out_cls, weights):
    """Host-side prep shared by kernel() and the timing harness: returns
    (w_scalars, in_maps)."""
    x = np.ascontiguousarray(x, dtype=np.float32)
    edge = np.ascontiguousarray(edge, dtype=np.float32)

    # x channel-major: [B, 128, 3, 784]; free order = (chunk, patch, quad),
    # patch = (i, j) row-major, quad = 2*r + cc
    nodes = x[:, 1:, :].reshape(B, HP, 2, HP, 2, C)
    xcm = (
        nodes.transpose(0, 5, 1, 3, 2, 4)        # [b, c, i, j, r, cc]
        .reshape(B, NCH, 128, HW)
        .transpose(0, 2, 1, 3)                   # [b, p, chunk, n]
        .astype(np.float16)
    )
    # edge channel-major: [B, 128, 3, 785]
    ecm = (
        edge.transpose(0, 2, 1)
        .reshape(B, NCH, 128, N)
        .transpose(0, 2, 1, 3)
        .astype(np.float16)
    )

    wlt = np.asarray(W_lin, dtype=np.float32).T          # [c, c']
    wlt_cm = (
        wlt.reshape(NCH, 128, C).transpose(1, 0, 2).astype(np.float16)
    )
    wct = np.asarray(W_out_cls, dtype=np.float32).T      # [c, co]
    wct_cm = (
        wct.reshape(NCH, 128, CO).transpose(1, 0, 2).astype(np.float16)
    )
    w = np.asarray(weights, dtype=np.float32)
    c_uniform = bool(np.all(w == w[0:1]))
    w_scalars = tuple(float(v) for v in w[0].reshape(4)) if c_uniform else None

    in_maps = []
    for core in range(NCORES):
        sl = slice(core * NB, (core + 1) * NB)
        cls_cm = (
            x[sl, 0, :].T.reshape(NCH, 128, NB)
            .transpose(1, 0, 2)
            .astype(np.float16)
        )
        m = {
            "x": xcm[sl], "edge": ecm[sl], "wlt": wlt_cm, "wct": wct_cm,
            "cls_cm": np.ascontiguousarray(cls_cm),
        }
        if w_scalars is None:
            wq = w.reshape(C, 4).reshape(NCH, 128, 4).transpose(1, 0, 2)
            m["wq_cm"] = np.ascontiguousarray(wq, dtype=np.float32)
        in_maps.append(m)
    return w_scalars, in_maps


def kernel(x, edge, W_lin, W_out_cls, weights):
    w_scalars, in_maps = prepare(x, edge, W_lin, W_out_cls, weights)
    nc = build_program(w_scalars)
    res = run_bass_kernel_spmd(nc, in_maps, list(range(NCORES)))
    out = np.concatenate([r["out"] for r in res.results], axis=0)
    return out


# revision 6
# speedup vs baseline: 1.5731x; 1.1034x over previous
"""Trainium2 Bass kernel for nn_Pooling_block (B=128, N=785, C=384, pp=2).

Pure data-parallel over batch: 16 batches per core x 8 NeuronCores.

v5 design ("c-major", fp16 inputs, balanced SDMA):
  - All large inputs sent to the device in float16 (end-to-end rel err
    ~1.4e-3, gate 2e-2), halving HBM read traffic vs f32: per-core reads
    ~20 MB, writes 9.7 MB -> ~83us floor @ 358 GB/s.
  - x host-pre-gathered CHANNEL-major [B, 128, 3, 784] (partition=channel
    mod 128); edge c-major in TWO token-halves [B, 128, 2, 1179] that are
    DMA-accumulated (CCE add) onto one SBUF tile, halving the DVE reduce.
  - All compute 128-partition-wide (c-major kills every PE transpose and
    every single-lane [1, N] op):
      * pair sums + node sums: 3 DVE stt ops with accum_out
      * edge sums: 1 DVE tensor_reduce over the pre-accumulated halves
      * group ci: 9 matmuls (W_lin.T blocks stationary)
      * scores: 6 matmuls with the ci column BROADCAST to [128,128] as
        stationary operand -> scores land replicated on all 128 partitions,
        so sigmoid/t01/pooled all run full-width; no broadcast matmul
      * pooled: one fp16 mul (t01 stride-0-broadcast across chunks) + one
        add writing fp16 straight into the final matmul operand acm
  - finals: 12 fp16 matmuls vs W_out_cls.T chunks; PSUM->SBUF on ACT.
  - Stores: [128, 768] rows via ACT HWDGE (even p//8 engine split), the
    69-row tail via gpsimd SWDGE whose swizzle spreads <128-partition
    transfers across all 16 SDMA engines (HWDGE puts 69 rows on 3).
"""
import os
import sys

sys.path.insert(0, "/opt/trn_rl_repo")

import numpy as np

import concourse.bass as bass
import concourse.tile as tile
from concourse import bacc, mybir
from concourse.bass_utils import run_bass_kernel_spmd

B, N, C = 128, 785, 384
HW = N - 1          # 784
H = 28              # grid side
HP = 14             # pooled grid side
NPATCH = HP * HP    # 196
NB = 16             # batches per core
NCORES = 8
NOUT = 1 + NPATCH   # 197
CO = 2 * C          # 768
NCH = 3             # channel chunks of 128
GRP = 4             # max batches per group
EHALF = 393         # edge tokens per accumulated half (2*393 = 786 >= 785)

F32 = mybir.dt.float32
F16 = mybir.dt.float16
ADD = mybir.AluOpType.add
MUL = mybir.AluOpType.mult
SIGMOID = mybir.ActivationFunctionType.Sigmoid
AXC = mybir.AxisListType.X


def build_program(w_scalars):
    """Build the per-core SPMD program. w_scalars = (w00, w01, w10, w11) when
    the per-patch weights are channel-uniform, else None (general path)."""
    nc = bacc.Bacc(None, target_bir_lowering=False, debug=False)

    x_d = nc.declare_dram_parameter("x", [NB, 128, NCH, HW], F16, isOutput=False)
    e_d = nc.declare_dram_parameter(
        "edge", [NB, 128, 2, NCH * EHALF], F16, isOutput=False
    )
    wlt_d = nc.declare_dram_parameter("wlt", [128, NCH, C], F16, isOutput=False)
    wct_d = nc.declare_dram_parameter("wct", [128, NCH, CO], F16, isOutput=False)
    clsc_d = nc.declare_dram_parameter("cls_cm", [128, NCH, NB], F16, isOutput=False)
    general_w = w_scalars is None
    if general_w:
        # cols 0-3: w[c, q]; cols 4-5: w[c,2j] + w[c,2j+1]
        wq_d = nc.declare_dram_parameter("wq_cm", [128, NCH, 6], F32, isOutput=False)
    out_d = nc.declare_dram_parameter("out", [NB, NOUT, CO], F32, isOutput=True)

    uniform_w = (not general_w) and len(set(w_scalars)) == 1

    with tile.TileContext(nc) as tc:
        with (
            tc.tile_pool(name="const", bufs=1) as cpool,
            tc.tile_pool(name="gx", bufs=6) as gxp,
            tc.tile_pool(name="ed", bufs=3) as edp,
            tc.tile_pool(name="apool", bufs=6) as ap,
            tc.tile_pool(name="work", bufs=2) as wk,
            tc.tile_pool(name="small", bufs=2) as sm,
            tc.tile_pool(name="sgp", bufs=2) as sgp,
            tc.tile_pool(name="acm", bufs=6) as acmp,
            tc.tile_pool(name="ost", bufs=4) as ostp,
            tc.tile_pool(name="psS", bufs=2, space="PSUM") as psS,
            tc.tile_pool(name="psC", bufs=1, space="PSUM") as psC,
            tc.tile_pool(name="psF", bufs=2, space="PSUM") as psF,
        ):
            # ---- constants ----
            wlt_h = cpool.tile([128, NCH, C], F16)
            nc.sync.dma_start(wlt_h[:], wlt_d[:])
            wct_h = cpool.tile([128, NCH, CO], F16)
            nc.sync.dma_start(wct_h[:], wct_d[:])
            cls_h = cpool.tile([128, NCH, NB], F16)
            nc.sync.dma_start(cls_h[:], clsc_d[:])
            if general_w:
                wq_h = cpool.tile([128, NCH, 6], F32)
                nc.sync.dma_start(wq_h[:], wq_d[:])

            group_list = [range(0, 2), range(2, 6), range(6, 10),
                          range(10, 14), range(14, 16)]
            for bs in group_list:
                glen = len(bs)
                x_t, a_t = {}, {}

                # s columns for the group, c-major: [128, chunk, batch]
                sG = sgp.tile([128, NCH, GRP], F16, tag="sG")

                # -- sub-loop 1: loads + per-batch token sums -> s columns --
                for b in bs:
                    gb = b - bs[0]
                    xb = gxp.tile([128, NCH, NPATCH, 4], F16, tag="xb")
                    nc.sync.dma_start(
                        xb[:],
                        x_d[b].rearrange("p k (n q) -> p k n q", q=4),
                    )
                    x_t[b] = xb
                    # edge halves DMA-accumulated: eb = half0 + half1
                    eb = edp.tile([128, NCH, EHALF], F16, tag="eb")
                    nc.sync.dma_start(
                        eb[:],
                        e_d[b, :, 0].rearrange("p (k t) -> p k t", k=NCH),
                    )
                    nc.gpsimd.dma_start(
                        eb[:],
                        e_d[b, :, 1].rearrange("p (k t) -> p k t", k=NCH),
                        accum_op=ADD,
                    )

                    # pair sums + node sums in one pass (3 stt ops w/ accum)
                    at = ap.tile([128, NCH, NPATCH, 2], F16, tag="at")
                    ns = sm.tile([128, NCH, 1], F32, tag="ns")
                    for k in range(NCH):
                        nc.vector.scalar_tensor_tensor(
                            at[:, k], xb[:, k, :, 0:2], 1.0, xb[:, k, :, 2:4],
                            MUL, ADD, accum_out=ns[:, k],
                        )
                    a_t[b] = at

                    es = sm.tile([128, NCH, 1], F32, tag="es")
                    nc.vector.tensor_reduce(es[:, :, 0], eb[:], AXC, ADD)

                    se = sm.tile([128, NCH, 1], F32, tag="se")
                    nc.scalar.activation(se[:], es[:], SIGMOID, scale=1.0 / N)
                    sn = sm.tile([128, NCH, 1], F32, tag="sn")
                    nc.scalar.activation(sn[:], ns[:], SIGMOID, scale=1.0 / HW)
                    nc.vector.tensor_add(sG[:, :, gb : gb + 1], se[:], sn[:])

                # -- group ci: 9 matmuls, all c-major, no transposes --
                ciG_ps = psC.tile([128, NCH, GRP], F32, tag="ciG")
                for m in range(NCH):
                    for k in range(NCH):
                        nc.tensor.matmul(
                            ciG_ps[:, m, 0:glen],
                            wlt_h[:, k, 128 * m : 128 * (m + 1)],
                            sG[:, k, 0:glen],
                            start=(k == 0), stop=(k == 2),
                        )
                ciG = sm.tile([128, NCH, GRP], F16, tag="ciG_sb")
                nc.scalar.copy(ciG[:], ciG_ps[:])

                # -- pass A: scores -> t01 -> pooled -> acm per batch --
                acms = {}
                for b in bs:
                    gb = b - bs[0]
                    xb, at = x_t[b], a_t[b]

                    # scores, replicated on all 128 partitions via a
                    # stride-0 broadcast of the ci column as lhsT
                    sigb = sm.tile([128, NPATCH, 2, 2], F16, tag="sigb")
                    for h in range(2):
                        sc_ps = psS.tile([128, 392], F32, tag="sc")
                        for k in range(NCH):
                            nc.tensor.matmul(
                                sc_ps[:],
                                ciG[:, k, gb : gb + 1].to_broadcast([128, 128]),
                                xb[:, k, 98 * h : 98 * (h + 1), :].rearrange(
                                    "p n q -> p (n q)"
                                ),
                                start=(k == 0), stop=(k == 2),
                            )
                        nc.scalar.activation(
                            sigb[:, 98 * h : 98 * (h + 1)].rearrange(
                                "p n j r -> p (n j r)"
                            ),
                            sc_ps[:], SIGMOID,
                        )

                    acm = acmp.tile([128, NCH, NOUT], F16, tag="acm")
                    nc.scalar.copy(acm[:, :, 0:1], cls_h[:, :, b : b + 1])
                    acms[b] = acm

                    # t01h tile has a singleton chunk dim for stride-0 bcast
                    t01h = wk.tile([128, 1, NPATCH, 2], F16, tag="t01h")
                    if not general_w:
                        # t01[p, j] = sig(4p+2j) + sig(4p+2j+1); fold weights:
                        # (t01 + 2) * w  (channel-uniform)
                        if uniform_w:
                            t01f = wk.tile([128, NPATCH, 2], F32, tag="t01f")
                            nc.vector.tensor_reduce(
                                t01f[:, :, :], sigb[:], AXC, ADD
                            )
                            w00 = w_scalars[0]
                            nc.vector.tensor_scalar(
                                t01h[:, 0], t01f[:], w00, 2.0 * w00, MUL, ADD
                            )
                        else:
                            wv = w_scalars
                            for j in range(2):
                                vj = wk.tile([128, NPATCH], F32, tag=f"vj{j}")
                                nc.vector.tensor_scalar(
                                    vj[:], sigb[:, :, j, 1],
                                    float(wv[2 * j + 1]),
                                    float(wv[2 * j] + wv[2 * j + 1]),
                                    MUL, ADD,
                                )
                                nc.vector.scalar_tensor_tensor(
                                    t01h[:, 0, :, j], sigb[:, :, j, 0],
                                    float(wv[2 * j]), vj[:], MUL, ADD,
                                )
                        prod = wk.tile([128, NCH, NPATCH, 2], F16, tag="prod")
                        nc.vector.tensor_mul(
                            prod[:], at[:],
                            t01h[:].to_broadcast([128, NCH, NPATCH, 2]),
                        )
                        nc.vector.tensor_add(
                            acm[:, :, 1:NOUT].rearrange(
                                "p k (n u) -> p k n u", u=1
                            ),
                            prod[:, :, :, 0:1], prod[:, :, :, 1:2],
                        )
                    else:
                        # fully general per-channel weights: m_j[c, p] =
                        # w[c,2j]*sig_j0 + w[c,2j+1]*sig_j1 + wsum_j[c]
                        for k in range(NCH):
                            mj = []
                            for j in range(2):
                                u = wk.tile([128, NPATCH], F32, tag=f"u{j}")
                                nc.vector.tensor_scalar_mul(
                                    u[:], sigb[:, :, j, 0],
                                    wq_h[:, k, 2 * j : 2 * j + 1],
                                )
                                m = wk.tile([128, NPATCH], F32, tag=f"m{j}")
                                nc.vector.scalar_tensor_tensor(
                                    m[:], sigb[:, :, j, 1],
                                    wq_h[:, k, 2 * j + 1 : 2 * j + 2],
                                    u[:], MUL, ADD,
                                )
                                m2 = wk.tile([128, NPATCH], F32, tag=f"m2{j}")
                                nc.vector.tensor_scalar(
                                    m2[:], m[:], wq_h[:, k, 4 + j : 5 + j],
                                    None, ADD,
                                )
                                mj.append(m2)
                            p0 = wk.tile([128, NPATCH], F32, tag="p0")
                            nc.vector.tensor_mul(
                                p0[:], at[:, k, :, 0], mj[0][:]
                            )
                            p1 = wk.tile([128, NPATCH], F32, tag="p1")
                            nc.vector.tensor_mul(
                                p1[:], at[:, k, :, 1], mj[1][:]
                            )
                            nc.vector.tensor_add(
                                acm[:, k, 1:NOUT], p0[:], p1[:]
                            )

                # -- pass B: finals + stores per batch --
                for b in bs:
                    acm = acms[b]
                    for r0, rn in ((0, 128), (128, 69)):
                        stile = ostp.tile([128, CO], F32, tag="ost")
                        for nh in range(2):
                            fo = psF.tile([128, C], F32, tag="fo")
                            for cch in range(NCH):
                                nc.tensor.matmul(
                                    fo[0:rn, :],
                                    acm[:, cch, r0 : r0 + rn],
                                    wct_h[:, cch, C * nh : C * (nh + 1)],
                                    start=(cch == 0), stop=(cch == 2),
                                )
                            nc.scalar.copy(
                                stile[0:rn, C * nh : C * (nh + 1)], fo[0:rn, :]
                            )
                        if rn == 128:
                            # 128-row store: HWDGE splits evenly (p//8)
                            nc.scalar.dma_start(
                                out_d[b, r0 : r0 + rn, :], stile[0:rn, :]
                            )
                        else:
                            # <128-row store: SWDGE swizzle spreads it across
                            # all 16 SDMA engines (HWDGE would use only 3)
                            nc.gpsimd.dma_start(
                                out_d[b, r0 : r0 + rn, :], stile[0:rn, :]
                            )

    nc.compile()
    return nc


def prepare(x, edge, W_lin, W_out_cls, weights):
    """Host-side prep shared by kernel() and the timing harness: returns
    (w_scalars, in_maps)."""
    x = np.ascontiguousarray(x, dtype=np.float32)
    edge = np.ascontiguousarray(edge, dtype=np.float32)

    # x channel-major: [B, 128, 3, 784]; free order = (chunk, patch, quad),
    # patch = (i, j) row-major, quad = 2*r + cc
    nodes = x[:, 1:, :].reshape(B, HP, 2, HP, 2, C)
    xcm = (
        nodes.transpose(0, 5, 1, 3, 2, 4)        # [b, c, i, j, r, cc]
        .reshape(B, NCH, 128, HW)
        .transpose(0, 2, 1, 3)                   # [b, p, chunk, n]
        .astype(np.float16)
    )
    # edge channel-major in two token-halves: [B, 128, 2, 3*393]
    ecm = edge.transpose(0, 2, 1)                # [b, c, t]
    eh = np.zeros((B, C, 2, EHALF), dtype=np.float16)
    eh[:, :, 0, :] = ecm[:, :, 0:EHALF]
    eh[:, :, 1, 0 : N - EHALF] = ecm[:, :, EHALF:N]
    ehd = (
        eh.reshape(B, NCH, 128, 2, EHALF)
        .transpose(0, 2, 3, 1, 4)                # [b, p, s, chunk, t]
        .reshape(B, 128, 2, NCH * EHALF)
    )

    wlt = np.asarray(W_lin, dtype=np.float32).T          # [c, c']
    wlt_cm = (
        wlt.reshape(NCH, 128, C).transpose(1, 0, 2).astype(np.float16)
    )
    wct = np.asarray(W_out_cls, dtype=np.float32).T      # [c, co]
    wct_cm = (
        wct.reshape(NCH, 128, CO).transpose(1, 0, 2).astype(np.float16)
    )
    w = np.asarray(weights, dtype=np.float32)
    c_uniform = bool(np.all(w == w[0:1]))
    w_scalars = tuple(float(v) for v in w[0].reshape(4)) if c_uniform else None

    in_maps = []
    for core in range(NCORES):
        sl = slice(core * NB, (core + 1) * NB)
        cls_cm = (
            x[sl, 0, :].T.reshape(NCH, 128, NB)
            .transpose(1, 0, 2)
            .astype(np.float16)
        )
        m = {
            "x": xcm[sl], "edge": ehd[sl], "wlt": wlt_cm, "wct": wct_cm,
            "cls_cm": np.ascontiguousarray(cls_cm),
        }
        if w_scalars is None:
            wq4 = w.reshape(C, 4)
            wq = np.concatenate(
                [wq4, wq4[:, 0:2].sum(1, keepdims=True),
                 wq4[:, 2:4].sum(1, keepdims=True)], axis=1
            )                                            # [C, 6]
            wq = wq.reshape(NCH, 128, 6).transpose(1, 0, 2)
            m["wq_cm"] = np.ascontiguousarray(wq, dtype=np.float32)
        in_maps.append(m)
    return w_scalars, in_maps


def kernel(x, edge, W_lin, W_out_cls, weights):
    w_scalars, in_maps = prepare(x, edge, W_lin, W_out_cls, weights)
    nc = build_program(w_scalars)
    res = run_bass_kernel_spmd(nc, in_maps, list(range(NCORES)))
    out = np.concatenate([r["out"] for r in res.results], axis=0)
    return out


# revision 7
# speedup vs baseline: 1.9737x; 1.2547x over previous
"""Trainium2 Bass kernel for nn_Pooling_block (B=128, N=785, C=384, pp=2).

Pure data-parallel over batch: 16 batches per core x 8 NeuronCores.

v6 design ("c-major", fp16 x / fp8 edge, balanced SDMA):
  - x in float16 channel-major [B, 128, 3, 784]; edge in float8-e4m3
    c-major [B, 128, 3, 788] (3 zero pad tokens).  Per-core HBM traffic:
    reads ~15.4 MB, writes 9.7 MB (the f32 output).  End-to-end rel err
    ~1.5e-3 (gate 2e-2) - fp8 only feeds a mean -> sigmoid, and the DVE
    accumulates it in f32.
  - all compute is 128-partition wide c-major (no PE transposes, no
    single-lane ops):
      * pair sums + node sums: 3 DVE stt ops with accum_out
      * edge sums: 3 DVE stt half-folds with accum_out (f32 accumulate)
      * one sigmoid per batch for both means ([128, 2, 3] tile; edge is
        host-pre-scaled by 784/785 so one scale fits both)
      * group ci: 9 matmuls (W_lin.T blocks stationary)
      * scores: 6 matmuls with the ci column stride-0-BROADCAST to
        [128,128] as the stationary operand -> scores land replicated on
        all 128 partitions
      * t01: ONE stt op: (sig0 + 2) + sig1 -> fp16
      * pooled: fp16 mul (t01 stride-0 broadcast across chunks) on DVE +
        final add on gpsimd writing fp16 into acm
  - finals: 12 fp16 matmuls vs W_out_cls.T chunks; PSUM->SBUF on ACT.
  - stores: [128, 768] rows via ACT HWDGE (even p//8 engine split), the
    69-row tail via gpsimd SWDGE whose swizzle spreads <128-partition
    transfers across all 16 SDMA engines (HWDGE would use only 3).
"""
import os
import sys

sys.path.insert(0, "/opt/trn_rl_repo")

import numpy as np
import ml_dtypes

import concourse.bass as bass
import concourse.tile as tile
from concourse import bacc, mybir
from concourse.bass_utils import run_bass_kernel_spmd

B, N, C = 128, 785, 384
HW = N - 1          # 784
H = 28              # grid side
HP = 14             # pooled grid side
NPATCH = HP * HP    # 196
NB = 16             # batches per core
NCORES = 8
NOUT = 1 + NPATCH   # 197
CO = 2 * C          # 768
NCH = 3             # channel chunks of 128
GRP = 4             # max batches per group
NE = 788            # padded edge tokens (2 * 394)
EH = NE // 2        # 394

F32 = mybir.dt.float32
F16 = mybir.dt.float16
F8 = mybir.dt.float8e4
ADD = mybir.AluOpType.add
MUL = mybir.AluOpType.mult
SIGMOID = mybir.ActivationFunctionType.Sigmoid
AXC = mybir.AxisListType.X


def build_program(w_scalars):
    """Build the per-core SPMD program. w_scalars = (w00, w01, w10, w11) when
    the per-patch weights are channel-uniform, else None (general path)."""
    nc = bacc.Bacc(None, target_bir_lowering=False, debug=False)

    x_d = nc.declare_dram_parameter("x", [NB, 128, NCH, HW], F16, isOutput=False)
    e_d = nc.declare_dram_parameter("edge", [NB, 128, NCH, NE], F8, isOutput=False)
    wlt_d = nc.declare_dram_parameter("wlt", [128, NCH, C], F16, isOutput=False)
    wct_d = nc.declare_dram_parameter("wct", [128, NCH, CO], F16, isOutput=False)
    clsc_d = nc.declare_dram_parameter("cls_cm", [128, NCH, NB], F16, isOutput=False)
    general_w = w_scalars is None
    if general_w:
        # cols 0-3: w[c, q]; cols 4-5: w[c,2j] + w[c,2j+1]
        wq_d = nc.declare_dram_parameter("wq_cm", [128, NCH, 6], F32, isOutput=False)
    out_d = nc.declare_dram_parameter("out", [NB, NOUT, CO], F32, isOutput=True)

    uniform_w = (not general_w) and len(set(w_scalars)) == 1

    with tile.TileContext(nc) as tc:
        with (
            tc.tile_pool(name="const", bufs=1) as cpool,
            tc.tile_pool(name="gx", bufs=8) as gxp,
            tc.tile_pool(name="ed", bufs=4) as edp,
            tc.tile_pool(name="apool", bufs=8) as ap,
            tc.tile_pool(name="work", bufs=4) as wk,
            tc.tile_pool(name="small", bufs=4) as sm,
            tc.tile_pool(name="sgp", bufs=2) as sgp,
            tc.tile_pool(name="acm", bufs=8) as acmp,
            tc.tile_pool(name="ost", bufs=6) as ostp,
            tc.tile_pool(name="psS", bufs=2, space="PSUM") as psS,
            tc.tile_pool(name="psC", bufs=1, space="PSUM") as psC,
            tc.tile_pool(name="psF", bufs=2, space="PSUM") as psF,
        ):
            wlt_h = cpool.tile([128, NCH, C], F16)
            wct_h = cpool.tile([128, NCH, CO], F16)
            cls_h = cpool.tile([128, NCH, NB], F16)
            if general_w:
                wq_h = cpool.tile([128, NCH, 6], F32)

            group_list = [range(0, 2), range(2, 6), range(6, 10),
                          range(10, 14), range(14, 15), range(15, 16)]
            first = True
            for bs in group_list:
                glen = len(bs)
                x_t, a_t = {}, {}

                # s columns for the group, c-major: [128, chunk, batch]
                sG = sgp.tile([128, NCH, GRP], F16, tag="sG")

                # -- sub-loop 1: loads + per-batch token sums -> s columns --
                for b in bs:
                    gb = b - bs[0]
                    xb = gxp.tile([128, NCH, NPATCH, 4], F16, tag="xb")
                    nc.sync.dma_start(
                        xb[:],
                        x_d[b].rearrange("p k (n q) -> p k n q", q=4),
                    )
                    x_t[b] = xb
                    eb = edp.tile([128, NCH, NE], F8, tag="eb")
                    nc.sync.dma_start(eb[:], e_d[b])

                    if first:
                        # weights go on the queue after the first batch's
                        # data so compute can ramp sooner; they're first
                        # needed by the group-ci matmuls
                        nc.sync.dma_start(wlt_h[:], wlt_d[:])
                        nc.sync.dma_start(wct_h[:], wct_d[:])
                        nc.sync.dma_start(cls_h[:], clsc_d[:])
                        if general_w:
                            nc.sync.dma_start(wq_h[:], wq_d[:])
                        first = False

                    # esns[:, 0, k] = edge sums, esns[:, 1, k] = node sums
                    esns = sm.tile([128, 2, NCH], F32, tag="esns")

                    # pair sums + node sums in one pass (3 stt ops w/ accum)
                    at = ap.tile([128, NCH, NPATCH, 2], F16, tag="at")
                    for k in range(NCH):
                        nc.vector.scalar_tensor_tensor(
                            at[:, k], xb[:, k, :, 0:2], 1.0, xb[:, k, :, 2:4],
                            MUL, ADD, accum_out=esns[:, 1, k : k + 1],
                        )
                    a_t[b] = at

                    # edge sums: fold the two token-halves, f32 accumulate
                    ef = wk.tile([128, NCH, EH], F16, tag="ef")
                    for k in range(NCH):
                        nc.vector.scalar_tensor_tensor(
                            ef[:, k], eb[:, k, 0:EH], 1.0, eb[:, k, EH:NE],
                            MUL, ADD, accum_out=esns[:, 0, k : k + 1],
                        )

                    # one sigmoid for both means (edge host-pre-scaled)
                    sg2 = sm.tile([128, 2, NCH], F32, tag="sg2")
                    nc.scalar.activation(
                        sg2[:], esns[:], SIGMOID, scale=1.0 / HW
                    )
                    nc.vector.tensor_add(
                        sG[:, :, gb : gb + 1],
                        sg2[:, 0:1, :].rearrange("p u k -> p k u"),
                        sg2[:, 1:2, :].rearrange("p u k -> p k u"),
                    )

                # -- group ci: 9 matmuls, all c-major, no transposes --
                ciG_ps = psC.tile([128, NCH, GRP], F32, tag="ciG")
                for m in range(NCH):
                    for k in range(NCH):
                        nc.tensor.matmul(
                            ciG_ps[:, m, 0:glen],
                            wlt_h[:, k, 128 * m : 128 * (m + 1)],
                            sG[:, k, 0:glen],
                            start=(k == 0), stop=(k == 2),
                        )
                ciG = sm.tile([128, NCH, GRP], F16, tag="ciG_sb")
                nc.scalar.copy(ciG[:], ciG_ps[:])

                # -- pass A: scores -> t01 -> pooled -> acm per batch --
                acms = {}
                for b in bs:
                    gb = b - bs[0]
                    xb, at = x_t[b], a_t[b]

                    # scores, replicated on all 128 partitions via a
                    # stride-0 broadcast of the ci column as lhsT
                    sigb = sm.tile([128, NPATCH, 2, 2], F16, tag="sigb")
                    for h in range(2):
                        sc_ps = psS.tile([128, 392], F32, tag="sc")
                        for k in range(NCH):
                            nc.tensor.matmul(
                                sc_ps[:],
                                ciG[:, k, gb : gb + 1].to_broadcast([128, 128]),
                                xb[:, k, 98 * h : 98 * (h + 1), :].rearrange(
                                    "p n q -> p (n q)"
                                ),
                                start=(k == 0), stop=(k == 2),
                            )
                        nc.scalar.activation(
                            sigb[:, 98 * h : 98 * (h + 1)].rearrange(
                                "p n j r -> p (n j r)"
                            ),
                            sc_ps[:], SIGMOID,
                        )

                    acm = acmp.tile([128, NCH, NOUT], F16, tag="acm")
                    nc.gpsimd.tensor_copy(acm[:, :, 0:1], cls_h[:, :, b : b + 1])
                    acms[b] = acm

                    # t01h tile has a singleton chunk dim for stride-0 bcast
                    t01h = wk.tile([128, 1, NPATCH, 2], F16, tag="t01h")
                    if not general_w:
                        if uniform_w and w_scalars[0] == 1.0:
                            # t01[p,j] = (sig(4p+2j) + 2) + sig(4p+2j+1)
                            nc.vector.scalar_tensor_tensor(
                                t01h[:, 0], sigb[:, :, :, 0:1], 2.0,
                                sigb[:, :, :, 1:2], ADD, ADD,
                            )
                        elif uniform_w:
                            w00 = w_scalars[0]
                            t01f = wk.tile([128, NPATCH, 2], F32, tag="t01f")
                            nc.vector.scalar_tensor_tensor(
                                t01f[:], sigb[:, :, :, 0:1], 2.0,
                                sigb[:, :, :, 1:2], ADD, ADD,
                            )
                            nc.vector.tensor_scalar(
                                t01h[:, 0], t01f[:], w00, None, MUL
                            )
                        else:
                            wv = w_scalars
                            for j in range(2):
                                vj = wk.tile([128, NPATCH], F32, tag=f"vj{j}")
                                nc.vector.tensor_scalar(
                                    vj[:], sigb[:, :, j, 1],
                                    float(wv[2 * j + 1]),
                                    float(wv[2 * j] + wv[2 * j + 1]),
                                    MUL, ADD,
                                )
                                nc.vector.scalar_tensor_tensor(
                                    t01h[:, 0, :, j], sigb[:, :, j, 0],
                                    float(wv[2 * j]), vj[:], MUL, ADD,
                                )
                        prod = wk.tile([128, NCH, NPATCH, 2], F16, tag="prod")
                        nc.vector.tensor_mul(
                            prod[:], at[:],
                            t01h[:].to_broadcast([128, NCH, NPATCH, 2]),
                        )
                        nc.gpsimd.tensor_add(
                            acm[:, :, 1:NOUT].rearrange(
                                "p k (n u) -> p k n u", u=1
                            ),
                            prod[:, :, :, 0:1], prod[:, :, :, 1:2],
                        )
                    else:
                        # fully general per-channel weights: m_j[c, p] =
                        # w[c,2j]*sig_j0 + w[c,2j+1]*sig_j1 + wsum_j[c]
                        for k in range(NCH):
                            mj = []
                            for j in range(2):
                                u = wk.tile([128, NPATCH], F32, tag=f"u{j}")
                                nc.vector.tensor_scalar_mul(
                                    u[:], sigb[:, :, j, 0],
                                    wq_h[:, k, 2 * j : 2 * j + 1],
                                )
                                m = wk.tile([128, NPATCH], F32, tag=f"m{j}")
                                nc.vector.scalar_tensor_tensor(
                                    m[:], sigb[:, :, j, 1],
                                    wq_h[:, k, 2 * j + 1 : 2 * j + 2],
                                    u[:], MUL, ADD,
                                )
                                m2 = wk.tile([128, NPATCH], F32, tag=f"m2{j}")
                                nc.vector.tensor_scalar(
                                    m2[:], m[:], wq_h[:, k, 4 + j : 5 + j],
                                    None, ADD,
                                )
                                mj.append(m2)
                            p0 = wk.tile([128, NPATCH], F32, tag="p0")
                            nc.vector.tensor_mul(
                                p0[:], at[:, k, :, 0], mj[0][:]
                            )
                            p1 = wk.tile([128, NPATCH], F32, tag="p1")
                            nc.vector.tensor_mul(
                                p1[:], at[:, k, :, 1], mj[1][:]
                            )
                            nc.vector.tensor_add(
                                acm[:, k, 1:NOUT], p0[:], p1[:]
                            )

                # -- pass B: finals + stores per batch --
                for b in bs:
                    acm = acms[b]
                    for r0, rn in ((0, 128), (128, 69)):
                        stile = ostp.tile([128, CO], F32, tag="ost")
                        for nh in range(2):
                            fo = psF.tile([128, C], F32, tag="fo")
                            for cch in range(NCH):
                                nc.tensor.matmul(
                                    fo[0:rn, :],
                                    acm[:, cch, r0 : r0 + rn],
                                    wct_h[:, cch, C * nh : C * (nh + 1)],
                                    start=(cch == 0), stop=(cch == 2),
                                )
                            nc.scalar.copy(
                                stile[0:rn, C * nh : C * (nh + 1)], fo[0:rn, :]
                            )
                        if rn == 128:
                            # 128-row store: HWDGE splits evenly (p//8)
                            nc.scalar.dma_start(
                                out_d[b, r0 : r0 + rn, :], stile[0:rn, :]
                            )
                        else:
                            # <128-row store: SWDGE swizzle spreads it across
                            # all 16 SDMA engines (HWDGE would use only 3)
                            nc.gpsimd.dma_start(
                                out_d[b, r0 : r0 + rn, :], stile[0:rn, :]
                            )

    nc.compile()
    return nc


def prepare(x, edge, W_lin, W_out_cls, weights):
    """Host-side prep shared by kernel() and the timing harness: returns
    (w_scalars, in_maps)."""
    x = np.ascontiguousarray(x, dtype=np.float32)
    edge = np.ascontiguousarray(edge, dtype=np.float32)

    # x channel-major: [B, 128, 3, 784]; free order = (chunk, patch, quad),
    # patch = (i, j) row-major, quad = 2*r + cc
    nodes = x[:, 1:, :].reshape(B, HP, 2, HP, 2, C)
    xcm = (
        nodes.transpose(0, 5, 1, 3, 2, 4)        # [b, c, i, j, r, cc]
        .reshape(B, NCH, 128, HW)
        .transpose(0, 2, 1, 3)                   # [b, p, chunk, n]
        .astype(np.float16)
    )
    # edge channel-major fp8, pre-scaled by 784/785 so the device can use a
    # single 1/784 sigmoid scale for both the edge and node means
    ecm = np.zeros((B, C, NE), dtype=np.float32)
    ecm[:, :, 0:N] = edge.transpose(0, 2, 1) * (float(HW) / N)
    ecm8 = (
        ecm.reshape(B, NCH, 128, NE)
        .transpose(0, 2, 1, 3)                   # [b, p, chunk, t]
        .astype(ml_dtypes.float8_e4m3)
    )

    wlt = np.asarray(W_lin, dtype=np.float32).T          # [c, c']
    wlt_cm = (
        wlt.reshape(NCH, 128, C).transpose(1, 0, 2).astype(np.float16)
    )
    wct = np.asarray(W_out_cls, dtype=np.float32).T      # [c, co]
    wct_cm = (
        wct.reshape(NCH, 128, CO).transpose(1, 0, 2).astype(np.float16)
    )
    w = np.asarray(weights, dtype=np.float32)
    c_uniform = bool(np.all(w == w[0:1]))
    w_scalars = tuple(float(v) for v in w[0].reshape(4)) if c_uniform else None

    in_maps = []
    for core in range(NCORES):
        sl = slice(core * NB, (core + 1) * NB)
        cls_cm = (
            x[sl, 0, :].T.reshape(NCH, 128, NB)
            .transpose(1, 0, 2)
            .astype(np.float16)
        )
        m = {
            "x": xcm[sl], "edge": ecm8[sl], "wlt": wlt_cm, "wct": wct_cm,
            "cls_cm": np.ascontiguousarray(cls_cm),
        }
        if w_scalars is None:
            wq4 = w.reshape(C, 4)
            wq = np.concatenate(
                [wq4, wq4[:, 0:2].sum(1, keepdims=True),
                 wq4[:, 2:4].sum(1, keepdims=True)], axis=1
            )                                            # [C, 6]
            wq = wq.reshape(NCH, 128, 6).transpose(1, 0, 2)
            m["wq_cm"] = np.ascontiguousarray(wq, dtype=np.float32)
        in_maps.append(m)
    return w_scalars, in_maps


def kernel(x, edge, W_lin, W_out_cls, weights):
    w_scalars, in_maps = prepare(x, edge, W_lin, W_out_cls, weights)
    nc = build_program(w_scalars)
    res = run_bass_kernel_spmd(nc, in_maps, list(range(NCORES)))
    out = np.concatenate([r["out"] for r in res.results], axis=0)
    return out


# revision 10
# speedup vs baseline: 1.9913x; 1.0089x over previous
"""Trainium2 Bass kernel for nn_Pooling_block (B=128, N=785, C=384, pp=2).

Pure data-parallel over batch: 16 batches per core x 8 NeuronCores.

v6 design ("c-major", fp16 x / fp8 edge, balanced SDMA):
  - x in float16 channel-major [B, 128, 3, 784]; edge in float8-e4m3
    c-major [B, 128, 3, 788] (3 zero pad tokens).  Per-core HBM traffic:
    reads ~15.4 MB, writes 9.7 MB (the f32 output).  End-to-end rel err
    ~1.5e-3 (gate 2e-2) - fp8 only feeds a mean -> sigmoid, and the DVE
    accumulates it in f32.
  - all compute is 128-partition wide c-major (no PE transposes, no
    single-lane ops):
      * pair sums + node sums: 3 DVE stt ops with accum_out
      * edge sums: 3 DVE stt half-folds with accum_out (f32 accumulate)
      * one sigmoid per batch for both means ([128, 2, 3] tile; edge is
        host-pre-scaled by 784/785 so one scale fits both)
      * group ci: 9 matmuls (W_lin.T blocks stationary)
      * scores: 6 matmuls with the ci column stride-0-BROADCAST to
        [128,128] as the stationary operand -> scores land replicated on
        all 128 partitions
      * t01: ONE stt op: (sig0 + 2) + sig1 -> fp16
      * pooled: fp16 mul (t01 stride-0 broadcast across chunks) on DVE +
        final add on gpsimd writing fp16 into acm
  - finals: 12 fp16 matmuls vs W_out_cls.T chunks; PSUM->SBUF on ACT.
  - stores: [128, 768] rows via ACT HWDGE (even p//8 engine split), the
    69-row tail via gpsimd SWDGE whose swizzle spreads <128-partition
    transfers across all 16 SDMA engines (HWDGE would use only 3).
"""
import os
import sys

sys.path.insert(0, "/opt/trn_rl_repo")

import numpy as np
import ml_dtypes

import concourse.bass as bass
import concourse.tile as tile
from concourse import bacc, mybir
from concourse.bass_utils import run_bass_kernel_spmd

B, N, C = 128, 785, 384
HW = N - 1          # 784
H = 28              # grid side
HP = 14             # pooled grid side
NPATCH = HP * HP    # 196
NB = 16             # batches per core
NCORES = 8
NOUT = 1 + NPATCH   # 197
CO = 2 * C          # 768
NCH = 3             # channel chunks of 128
GRP = 4             # max batches per group
NE = 788            # padded edge tokens (2 * 394)
EH = NE // 2        # 394

F32 = mybir.dt.float32
F16 = mybir.dt.float16
F8 = mybir.dt.float8e4
ADD = mybir.AluOpType.add
MUL = mybir.AluOpType.mult
SIGMOID = mybir.ActivationFunctionType.Sigmoid
AXC = mybir.AxisListType.X


def build_program(w_scalars):
    """Build the per-core SPMD program. w_scalars = (w00, w01, w10, w11) when
    the per-patch weights are channel-uniform, else None (general path)."""
    nc = bacc.Bacc(None, target_bir_lowering=False, debug=False)

    x_d = nc.declare_dram_parameter("x", [NB, 128, NCH, HW], F16, isOutput=False)
    e_d = nc.declare_dram_parameter("edge", [NB, 128, NCH, NE], F8, isOutput=False)
    wlt_d = nc.declare_dram_parameter("wlt", [128, NCH, C], F16, isOutput=False)
    wct_d = nc.declare_dram_parameter("wct", [128, NCH, CO], F16, isOutput=False)
    clsc_d = nc.declare_dram_parameter("cls_cm", [128, NCH, NB], F16, isOutput=False)
    general_w = w_scalars is None
    if general_w:
        # cols 0-3: w[c, q]; cols 4-5: w[c,2j] + w[c,2j+1]
        wq_d = nc.declare_dram_parameter("wq_cm", [128, NCH, 6], F32, isOutput=False)
    out_d = nc.declare_dram_parameter("out", [NB, NOUT, CO], F32, isOutput=True)

    uniform_w = (not general_w) and len(set(w_scalars)) == 1

    with tile.TileContext(nc) as tc:
        with (
            tc.tile_pool(name="const", bufs=1) as cpool,
            tc.tile_pool(name="gx", bufs=8) as gxp,
            tc.tile_pool(name="ed", bufs=4) as edp,
            tc.tile_pool(name="apool", bufs=8) as ap,
            tc.tile_pool(name="work", bufs=4) as wk,
            tc.tile_pool(name="small", bufs=4) as sm,
            tc.tile_pool(name="sgp", bufs=2) as sgp,
            tc.tile_pool(name="acm", bufs=8) as acmp,
            tc.tile_pool(name="ost", bufs=6) as ostp,
            tc.tile_pool(name="psS", bufs=2, space="PSUM") as psS,
            tc.tile_pool(name="psC", bufs=1, space="PSUM") as psC,
            tc.tile_pool(name="psF", bufs=2, space="PSUM") as psF,
        ):
            wlt_h = cpool.tile([128, NCH, C], F16)
            wct_h = cpool.tile([128, NCH, CO], F16)
            cls_h = cpool.tile([128, NCH, NB], F16)
            if general_w:
                wq_h = cpool.tile([128, NCH, 6], F32)

            group_list = [range(0, 1), range(1, 2), range(2, 6), range(6, 10),
                          range(10, 14), range(14, 15), range(15, 16)]
            first = True
            for bs in group_list:
                glen = len(bs)
                x_t, a_t = {}, {}

                # s columns for the group, c-major: [128, chunk, batch]
                sG = sgp.tile([128, NCH, GRP], F16, tag="sG")

                # -- sub-loop 1: loads + per-batch token sums -> s columns --
                for b in bs:
                    gb = b - bs[0]
                    xb = gxp.tile([128, NCH, NPATCH, 4], F16, tag="xb")
                    nc.sync.dma_start(
                        xb[:],
                        x_d[b].rearrange("p k (n q) -> p k n q", q=4),
                    )
                    x_t[b] = xb
                    eb = edp.tile([128, NCH, NE], F8, tag="eb")
                    nc.sync.dma_start(eb[:], e_d[b])

                    if first:
                        # weights go on the queue after the first batch's
                        # data so compute can ramp sooner; they're first
                        # needed by the group-ci matmuls
                        nc.sync.dma_start(wlt_h[:], wlt_d[:])
                        nc.sync.dma_start(wct_h[:], wct_d[:])
                        nc.sync.dma_start(cls_h[:], clsc_d[:])
                        if general_w:
                            nc.sync.dma_start(wq_h[:], wq_d[:])
                        first = False

                    # esns[:, 0, k] = edge sums, esns[:, 1, k] = node sums
                    esns = sm.tile([128, 2, NCH], F32, tag="esns")

                    # pair sums + node sums in one pass (3 stt ops w/ accum)
                    at = ap.tile([128, NCH, NPATCH, 2], F16, tag="at")
                    for k in range(NCH):
                        nc.vector.scalar_tensor_tensor(
                            at[:, k], xb[:, k, :, 0:2], 1.0, xb[:, k, :, 2:4],
                            MUL, ADD, accum_out=esns[:, 1, k : k + 1],
                        )
                    a_t[b] = at

                    # edge sums: fold the two token-halves, f32 accumulate
                    ef = wk.tile([128, NCH, EH], F16, tag="ef")
                    for k in range(NCH):
                        nc.vector.scalar_tensor_tensor(
                            ef[:, k], eb[:, k, 0:EH], 1.0, eb[:, k, EH:NE],
                            MUL, ADD, accum_out=esns[:, 0, k : k + 1],
                        )

                    # one sigmoid for both means (edge host-pre-scaled)
                    sg2 = sm.tile([128, 2, NCH], F32, tag="sg2")
                    nc.scalar.activation(
                        sg2[:], esns[:], SIGMOID, scale=1.0 / HW
                    )
                    nc.vector.tensor_add(
                        sG[:, :, gb : gb + 1],
                        sg2[:, 0:1, :].rearrange("p u k -> p k u"),
                        sg2[:, 1:2, :].rearrange("p u k -> p k u"),
                    )

                # -- group ci: 9 matmuls, all c-major, no transposes --
                ciG_ps = psC.tile([128, NCH, GRP], F32, tag="ciG")
                for m in range(NCH):
                    for k in range(NCH):
                        nc.tensor.matmul(
                            ciG_ps[:, m, 0:glen],
                            wlt_h[:, k, 128 * m : 128 * (m + 1)],
                            sG[:, k, 0:glen],
                            start=(k == 0), stop=(k == 2),
                        )
                ciG = sm.tile([128, NCH, GRP], F16, tag="ciG_sb")
                nc.scalar.copy(ciG[:], ciG_ps[:])

                # -- pass A: scores -> t01 -> pooled -> acm per batch --
                acms = {}
                for b in bs:
                    gb = b - bs[0]
                    xb, at = x_t[b], a_t[b]

                    # scores, replicated on all 128 partitions via a
                    # stride-0 broadcast of the ci column as lhsT
                    sigb = sm.tile([128, NPATCH, 2, 2], F16, tag="sigb")
                    for h in range(2):
                        sc_ps = psS.tile([128, 392], F32, tag="sc")
                        for k in range(NCH):
                            nc.tensor.matmul(
                                sc_ps[:],
                                ciG[:, k, gb : gb + 1].to_broadcast([128, 128]),
                                xb[:, k, 98 * h : 98 * (h + 1), :].rearrange(
                                    "p n q -> p (n q)"
                                ),
                                start=(k == 0), stop=(k == 2),
                            )
                        nc.scalar.activation(
                            sigb[:, 98 * h : 98 * (h + 1)].rearrange(
                                "p n j r -> p (n j r)"
                            ),
                            sc_ps[:], SIGMOID,
                        )

                    acm = acmp.tile([128, NCH, NOUT], F16, tag="acm")
                    nc.gpsimd.tensor_copy(acm[:, :, 0:1], cls_h[:, :, b : b + 1])
                    acms[b] = acm

                    # t01h tile has a singleton chunk dim for stride-0 bcast
                    t01h = wk.tile([128, 1, NPATCH, 2], F16, tag="t01h")
                    if not general_w:
                        if uniform_w and w_scalars[0] == 1.0:
                            # t01[p,j] = (sig(4p+2j) + 2) + sig(4p+2j+1)
                            nc.vector.scalar_tensor_tensor(
                                t01h[:, 0], sigb[:, :, :, 0:1], 2.0,
                                sigb[:, :, :, 1:2], ADD, ADD,
                            )
                        elif uniform_w:
                            w00 = w_scalars[0]
                            t01f = wk.tile([128, NPATCH, 2], F32, tag="t01f")
                            nc.vector.scalar_tensor_tensor(
                                t01f[:], sigb[:, :, :, 0:1], 2.0,
                                sigb[:, :, :, 1:2], ADD, ADD,
                            )
                            nc.vector.tensor_scalar(
                                t01h[:, 0], t01f[:], w00, None, MUL
                            )
                        else:
                            wv = w_scalars
                            for j in range(2):
                                vj = wk.tile([128, NPATCH], F32, tag=f"vj{j}")
                                nc.vector.tensor_scalar(
                                    vj[:], sigb[:, :, j, 1],
                                    float(wv[2 * j + 1]),
                                    float(wv[2 * j] + wv[2 * j + 1]),
                                    MUL, ADD,
                                )
                                nc.vector.scalar_tensor_tensor(
                                    t01h[:, 0, :, j], sigb[:, :, j, 0],
                                    float(wv[2 * j]), vj[:], MUL, ADD,
                                )
                        prod = wk.tile([128, NCH, NPATCH, 2], F16, tag="prod")
                        nc.vector.tensor_mul(
                            prod[:], at[:],
                            t01h[:].to_broadcast([128, NCH, NPATCH, 2]),
                        )
                        # acm add off DVE (the global bottleneck) except for
                        # the tail batches, where gpsimd's slow op + the
                        # cross-engine hop would sit on the critical path
                        acm_eng = nc.vector if b >= NB - 2 else nc.gpsimd
                        acm_eng.tensor_add(
                            acm[:, :, 1:NOUT].rearrange(
                                "p k (n u) -> p k n u", u=1
                            ),
                            prod[:, :, :, 0:1], prod[:, :, :, 1:2],
                        )
                    else:
                        # fully general per-channel weights: m_j[c, p] =
                        # w[c,2j]*sig_j0 + w[c,2j+1]*sig_j1 + wsum_j[c]
                        for k in range(NCH):
                            mj = []
                            for j in range(2):
                                u = wk.tile([128, NPATCH], F32, tag=f"u{j}")
                                nc.vector.tensor_scalar_mul(
                                    u[:], sigb[:, :, j, 0],
                                    wq_h[:, k, 2 * j : 2 * j + 1],
                                )
                                m = wk.tile([128, NPATCH], F32, tag=f"m{j}")
                                nc.vector.scalar_tensor_tensor(
                                    m[:], sigb[:, :, j, 1],
                                    wq_h[:, k, 2 * j + 1 : 2 * j + 2],
                                    u[:], MUL, ADD,
                                )
                                m2 = wk.tile([128, NPATCH], F32, tag=f"m2{j}")
                                nc.vector.tensor_scalar(
                                    m2[:], m[:], wq_h[:, k, 4 + j : 5 + j],
                                    None, ADD,
                                )
                                mj.append(m2)
                            p0 = wk.tile([128, NPATCH], F32, tag="p0")
                            nc.vector.tensor_mul(
                                p0[:], at[:, k, :, 0], mj[0][:]
                            )
                            p1 = wk.tile([128, NPATCH], F32, tag="p1")
                            nc.vector.tensor_mul(
                                p1[:], at[:, k, :, 1], mj[1][:]
                            )
                            nc.vector.tensor_add(
                                acm[:, k, 1:NOUT], p0[:], p1[:]
                            )

                # -- pass B: finals + stores per batch --
                for b in bs:
                    acm = acms[b]
                    for r0, rn in ((0, 128), (128, 69)):
                        stile = ostp.tile([128, CO], F32, tag="ost")
                        for nh in range(2):
                            fo = psF.tile([128, C], F32, tag="fo")
                            for cch in range(NCH):
                                nc.tensor.matmul(
                                    fo[0:rn, :],
                                    acm[:, cch, r0 : r0 + rn],
                                    wct_h[:, cch, C * nh : C * (nh + 1)],
                                    start=(cch == 0), stop=(cch == 2),
                                )
                            nc.scalar.copy(
                                stile[0:rn, C * nh : C * (nh + 1)], fo[0:rn, :]
                            )
                        if rn == 128:
                            # 128-row store: HWDGE splits evenly (p//8);
                            # issued from sync to keep ACT free for copies
                            nc.sync.dma_start(
                                out_d[b, r0 : r0 + rn, :], stile[0:rn, :]
                            )
                        else:
                            # <128-row store: SWDGE swizzle spreads it across
                            # all 16 SDMA engines (HWDGE would use only 3)
                            nc.gpsimd.dma_start(
                                out_d[b, r0 : r0 + rn, :], stile[0:rn, :]
                            )

    nc.compile()
    return nc


def prepare(x, edge, W_lin, W_out_cls, weights):
    """Host-side prep shared by kernel() and the timing harness: returns
    (w_scalars, in_maps)."""
    x = np.ascontiguousarray(x, dtype=np.float32)
    edge = np.ascontiguousarray(edge, dtype=np.float32)

    # x channel-major: [B, 128, 3, 784]; free order = (chunk, patch, quad),
    # patch = (i, j) row-major, quad = 2*r + cc
    nodes = x[:, 1:, :].reshape(B, HP, 2, HP, 2, C)
    xcm = (
        nodes.transpose(0, 5, 1, 3, 2, 4)        # [b, c, i, j, r, cc]
        .reshape(B, NCH, 128, HW)
        .transpose(0, 2, 1, 3)                   # [b, p, chunk, n]
        .astype(np.float16)
    )
    # edge channel-major fp8, pre-scaled by 784/785 so the device can use a
    # single 1/784 sigmoid scale for both the edge and node means
    ecm = np.zeros((B, C, NE), dtype=np.float32)
    ecm[:, :, 0:N] = edge.transpose(0, 2, 1) * (float(HW) / N)
    ecm8 = (
        ecm.reshape(B, NCH, 128, NE)
        .transpose(0, 2, 1, 3)                   # [b, p, chunk, t]
        .astype(ml_dtypes.float8_e4m3)
    )

    wlt = np.asarray(W_lin, dtype=np.float32).T          # [c, c']
    wlt_cm = (
        wlt.reshape(NCH, 128, C).transpose(1, 0, 2).astype(np.float16)
    )
    wct = np.asarray(W_out_cls, dtype=np.float32).T      # [c, co]
    wct_cm = (
        wct.reshape(NCH, 128, CO).transpose(1, 0, 2).astype(np.float16)
    )
    w = np.asarray(weights, dtype=np.float32)
    c_uniform = bool(np.all(w == w[0:1]))
    w_scalars = tuple(float(v) for v in w[0].reshape(4)) if c_uniform else None

    in_maps = []
    for core in range(NCORES):
        sl = slice(core * NB, (core + 1) * NB)
        cls_cm = (
            x[sl, 0, :].T.reshape(NCH, 128, NB)
            .transpose(1, 0, 2)
            .astype(np.float16)
        )
        m = {
            "x": xcm[sl], "edge": ecm8[sl], "wlt": wlt_cm, "wct": wct_cm,
            "cls_cm": np.ascontiguousarray(cls_cm),
        }
        if w_scalars is None:
            wq4 = w.reshape(C, 4)
            wq = np.concatenate(
                [wq4, wq4[:, 0:2].sum(1, keepdims=True),
                 wq4[:, 2:4].sum(1, keepdims=True)], axis=1
            )                                            # [C, 6]
            wq = wq.reshape(NCH, 128, 6).transpose(1, 0, 2)
            m["wq_cm"] = np.ascontiguousarray(wq, dtype=np.float32)
        in_maps.append(m)
    return w_scalars, in_maps


def kernel(x, edge, W_lin, W_out_cls, weights):
    w_scalars, in_maps = prepare(x, edge, W_lin, W_out_cls, weights)
    nc = build_program(w_scalars)
    res = run_bass_kernel_spmd(nc, in_maps, list(range(NCORES)))
    out = np.concatenate([r["out"] for r in res.results], axis=0)
    return out
